# revision 13
# baseline (speedup 1.0000x reference)
"""MoE routing kernel for TRN2, 8 NeuronCores (expert-parallel).

Math: out[t] = sum_{e in top2(logits[t])} x[t] @ w_up[e] @ w_down[e]
(reference applies no activation between projections and no prob weighting,
so each expert collapses to one fused matrix W_e = w_up[e] @ w_down[e]).

Distribution:
  phase 1 (expert parallel): core c computes W_c = w_up[c] @ w_down[c]
           in bf16 (fp32 PSUM accumulation), 4.3 G MACs/core.
  AllGather of the fused W matrices (bf16), split into two d2-halves so
           the collective overlaps phase-1 tail and apply head.
  phase 2 (token parallel): core c owns tokens [256c, 256(c+1)); computes
           fp32 routing logits, top-2 mask, and accumulates the 8 masked
           matmuls x_loc @ W_e into PSUM. Output is an exact token shard.
"""

import numpy as np

E = 8
TOPK = 2
D = 1024
I = 4096
T = 2048  # 4*512 tokens
N_CORES = 8
TL = T // N_CORES  # tokens per core (256)
P = 128
BIG = 1.0e30

_cached = {}


def _build():
    import concourse.bass as bass  # noqa: F401
    import concourse.tile as tile
    from concourse import bacc, mybir
    from concourse.masks import make_identity

    f32 = mybir.dt.float32
    bf16 = mybir.dt.bfloat16

    nc = bacc.Bacc("TRN2", target_bir_lowering=False, debug=False, num_devices=N_CORES)
    x_ext = nc.declare_dram_parameter("hidden_states", [TL, D], f32, isOutput=False)
    gate_ext = nc.declare_dram_parameter("gate_w", [E, D], f32, isOutput=False)
    up_ext = nc.declare_dram_parameter("w_up", [D, I], f32, isOutput=False)
    down_ext = nc.declare_dram_parameter("w_down", [I, D], f32, isOutput=False)
    out_ext = nc.declare_dram_parameter("out", [TL, D], f32, isOutput=True)

    KB = D // P      # 8  d1 blocks
    KI = I // P      # 32 i blocks
    TB = TL // P     # 2  token blocks
    NCH = D // 512   # 2  d2 chunks

    with tile.TileContext(nc) as tc:
        import contextlib

        with contextlib.ExitStack() as ctx:
            # ---- long-lived small pools ----
            const = ctx.enter_context(tc.tile_pool(name="const", bufs=1))
            outer = ctx.enter_context(tc.tile_pool(name="outer", bufs=1))
            dram = ctx.enter_context(tc.tile_pool(name="dram", bufs=1, space="DRAM"))

            ident = const.tile([P, P], f32)
            make_identity(nc, ident[:])
            ones1 = const.tile([1, P], f32)
            nc.vector.memset(ones1[:], 1.0)

            xT = outer.tile([P, KB, TL], f32)        # x_loc^T  [d1, t]
            mbc = outer.tile([P, E, TL], f32)        # per-expert masks bcast over partitions
            maskT = outer.tile([1, E, TL], f32)      # mask^T rows, all on partition 0
            mtmp = outer.tile([8, TB, P], f32)
            logits = outer.tile([P, TB, E], f32)
            m1 = outer.tile([P, TB], f32)
            eqbig = outer.tile([P, TB, E], f32)
            l2 = outer.tile([P, TB, E], f32)
            m2 = outer.tile([P, TB], f32)
            mask = outer.tile([P, TB, E], f32)
            gate_wT = outer.tile([P, KB, E], f32)

            # AG bounce buffers, one per d2 half; free dims flattened so every
            # DMA run is >=4KB contiguous per partition.
            ag_in = []
            ag_out = []
            for h in range(NCH):
                gi = dram.tile([P, KB * 512], bf16, name=f"ag_in_{h}")
                go = dram.tile(
                    [E, P, KB * 512], bf16, addr_space="Shared", name=f"ag_out_{h}"
                )
                ag_in.append(gi)
                ag_out.append(go)

            # ---- big weight pools (phase 1) ----
            big_cm = tc.tile_pool(name="big", bufs=1)
            big = big_cm.__enter__()
            upT = big.tile([P, KI, D], bf16)   # up^T  [i, d1]
            down = big.tile([P, KI, D], bf16)  # down  [i, d2]

            # ---- transient phase-0/1 pool ----
            ph1_cm = tc.tile_pool(name="ph1", bufs=3)
            ph1 = ph1_cm.__enter__()
            psA_cm = tc.tile_pool(name="psA", bufs=4, space="PSUM")
            psA = psA_cm.__enter__()

            # loads
            xnat = ph1.tile([P, TB, D], f32, tag="xnat", bufs=1)
            nc.sync.dma_start(xnat[:], x_ext.rearrange("(b p) d -> p b d", p=P))
            gsb = ph1.tile([8, D], f32, tag="gate", bufs=1)
            nc.sync.dma_start(gsb[:], gate_ext[:])

            # w_down cast-load (f32 -> bf16 via SWDGE), 4 quarters, 4KB runs
            for q in range(4):
                nc.gpsimd.dma_start(
                    down[:, 8 * q : 8 * (q + 1), :],
                    down_ext[1024 * q : 1024 * (q + 1), :].rearrange(
                        "(ko p) n -> p ko n", p=P
                    ),
                )

            # w_up cast-load natural slabs (16KB runs) + SBUF->SBUF xbar
            # transposes (off the PE).  Slab b covers d1 rows [128b,128b+128).
            upslabs = []
            for b in range(KB):
                us = ph1.tile([P, I], bf16, tag="upslab", name=f"upslab_{b}")
                nc.gpsimd.dma_start(us[:], up_ext[P * b : P * (b + 1), :])
                upslabs.append(us)
            for b in range(KB):
                us = upslabs[b]
                for k in range(KI):
                    eng = nc.sync
                    eng.dma_start_transpose(
                        upT[:, k, P * b : P * (b + 1)],
                        us[:, P * k : P * (k + 1)],
                    )

            # gate_w transpose: [8, 1024] -> [1024, 8]
            for kb in range(KB):
                pt = psA.tile([P, 8], f32, tag="tp")
                nc.tensor.transpose(pt[:], gsb[:, P * kb : P * (kb + 1)], ident[:8, :8])
                nc.any.tensor_copy(out=gate_wT[:, kb, :], in_=pt[:])

            # x transpose: [256, 1024] -> [1024, 256]
            for tb in range(TB):
                for kb in range(KB):
                    pt = psA.tile([P, P], f32, tag="tp")
                    nc.tensor.transpose(
                        pt[:], xnat[:, tb, P * kb : P * (kb + 1)], ident[:]
                    )
                    nc.any.tensor_copy(
                        out=xT[:, kb, P * tb : P * (tb + 1)], in_=pt[:]
                    )

            # router logits (fp32 exact): logits[t, e] = x @ gate_w.T
            for tb in range(TB):
                pl = psA.tile([P, E], f32, tag="tp")
                for kb in range(KB):
                    nc.tensor.matmul(
                        pl[:],
                        xT[:, kb, P * tb : P * (tb + 1)],
                        gate_wT[:, kb, :],
                        start=(kb == 0),
                        stop=(kb == KB - 1),
                    )
                nc.any.tensor_copy(out=logits[:, tb, :], in_=pl[:])

            # top-2 mask: mask = (l >= second_max(l))
            nc.vector.tensor_reduce(
                m1[:], logits[:], axis=mybir.AxisListType.X, op=mybir.AluOpType.max
            )
            nc.vector.tensor_tensor(
                eqbig[:],
                logits[:],
                m1[:, :, None].to_broadcast([P, TB, E]),
                mybir.AluOpType.is_equal,
            )
            nc.vector.tensor_scalar_mul(eqbig[:], eqbig[:], BIG)
            nc.vector.tensor_tensor(
                l2[:], logits[:], eqbig[:], mybir.AluOpType.subtract
            )
            nc.vector.tensor_reduce(
                m2[:], l2[:], axis=mybir.AxisListType.X, op=mybir.AluOpType.max
            )
            nc.vector.tensor_tensor(
                mask[:],
                logits[:],
                m2[:, :, None].to_broadcast([P, TB, E]),
                mybir.AluOpType.is_ge,
            )

            # mask^T: [256, 8] -> [8, 256], then DMA rows onto partition 0
            for tb in range(TB):
                pt = psA.tile([P, P], f32, tag="tp")
                nc.tensor.transpose(pt[:8, :], mask[:, tb, :], ident[:])
                nc.any.tensor_copy(out=mtmp[:, tb, :], in_=pt[:8, :])
            for tb in range(TB):
                nc.sync.dma_start(
                    maskT[0:1, :, P * tb : P * (tb + 1)], mtmp[:, tb, :]
                )

            # mask broadcast tiles via PE outer product (ones^T x maskT[e])
            for e in range(E):
                pb = psA.tile([P, TL], f32, tag="tp")
                nc.tensor.matmul(
                    pb[:], ones1[:], maskT[0:1, e, :], start=True, stop=True
                )
                nc.any.tensor_copy(out=mbc[:, e, :], in_=pb[:])

            psA_cm.__exit__(None, None, None)

            # ---- phase 1 matmuls: W_c = up^T.T @ down  (bf16) ----
            # nch-outer so the d2-half AllGather can fire at phase-1 midpoint.
            psW_cm = tc.tile_pool(name="psW", bufs=4, space="PSUM")
            psW = psW_cm.__enter__()
            for nch in range(NCH):
                for mg in range(2):
                    pw = [
                        psW.tile([P, 512], f32, tag="w", name=f"pw_{nch}_{mg}_{j}")
                        for j in range(4)
                    ]
                    for k in range(KI):
                        for m4 in range(4):
                            m = 4 * mg + m4
                            nc.tensor.matmul(
                                pw[m4][:],
                                upT[:, k, P * m : P * (m + 1)],
                                down[:, k, 512 * nch : 512 * (nch + 1)],
                                start=(k == 0),
                                stop=(k == KI - 1),
                            )
                    wev = ph1.tile([P, 4, 512], bf16, tag="wev")
                    for m4 in range(4):
                        nc.any.tensor_copy(out=wev[:, m4, :], in_=pw[m4][:])
                    nc.sync.dma_start(
                        ag_in[nch][:, 2048 * mg : 2048 * (mg + 1)], wev[:]
                    )
                # ---- AllGather for this d2 half ----
                nc.gpsimd.collective_compute(
                    "AllGather",
                    mybir.AluOpType.bypass,
                    replica_groups=[list(range(N_CORES))],
                    ins=[ag_in[nch].opt()],
                    outs=[ag_out[nch].opt()],
                )
            psW_cm.__exit__(None, None, None)

            # close phase-1 pools; open apply pools
            ph1_cm.__exit__(None, None, None)
            big_cm.__exit__(None, None, None)

            ap_cm = tc.tile_pool(name="apply", bufs=3)
            ap = ap_cm.__enter__()
            xmp_cm = tc.tile_pool(name="xm", bufs=1)
            xmp = xmp_cm.__enter__()
            psO_cm = tc.tile_pool(name="psO", bufs=4, space="PSUM")
            psO = psO_cm.__enter__()

            pout = [
                psO.tile([P, 512], f32, tag="o", name=f"pout_{j}") for j in range(4)
            ]
            xms = []

            for nch in range(NCH):
                for e in range(E):
                    we = ap.tile([P, KB * 512], bf16, tag="wstr", name=f"we_{nch}_{e}")
                    nc.sync.dma_start(we[:], ag_out[nch][e])
                    if nch == 0:
                        xm = xmp.tile([P, KB, TL], bf16, tag=f"xm{e}", name=f"xm_{e}")
                        for kb in range(KB):
                            nc.vector.tensor_tensor(
                                xm[:, kb, :],
                                xT[:, kb, :],
                                mbc[:, e, :],
                                mybir.AluOpType.mult,
                            )
                        xms.append(xm)
                    xm = xms[e]
                    for mt in range(TB):
                        for kb in range(KB):
                            nc.tensor.matmul(
                                pout[2 * mt + nch][:],
                                xm[:, kb, P * mt : P * (mt + 1)],
                                we[:, 512 * kb : 512 * (kb + 1)],
                                start=(e == 0 and kb == 0),
                                stop=(e == E - 1 and kb == KB - 1),
                            )

            outsb = ap.tile([P, TB, D], f32, tag="outsb", bufs=1)
            for mt in range(TB):
                for nch in range(NCH):
                    nc.any.tensor_copy(
                        out=outsb[:, mt, 512 * nch : 512 * (nch + 1)],
                        in_=pout[2 * mt + nch][:],
                    )
            nc.sync.dma_start(
                out_ext.rearrange("(b p) d -> p b d", p=P), outsb[:]
            )

            psO_cm.__exit__(None, None, None)
            xmp_cm.__exit__(None, None, None)
            ap_cm.__exit__(None, None, None)

    nc.finalize()
    return nc


def _get_nc():
    if "nc" not in _cached:
        _cached["nc"] = _build()
    return _cached["nc"]


def _make_in_maps(inputs):
    hs = np.asarray(inputs["hidden_states"], dtype=np.float32)
    gate_w = np.ascontiguousarray(np.asarray(inputs["gate_w"], dtype=np.float32))
    w_up = np.asarray(inputs["w_up"], dtype=np.float32)
    w_down = np.asarray(inputs["w_down"], dtype=np.float32)
    x = np.ascontiguousarray(hs.reshape(-1, D))
    in_maps = []
    for c in range(N_CORES):
        in_maps.append(
            {
                "hidden_states": np.ascontiguousarray(x[TL * c : TL * (c + 1)]),
                "gate_w": gate_w,
                "w_up": np.ascontiguousarray(w_up[c]),
                "w_down": np.ascontiguousarray(w_down[c]),
            }
        )
    return in_maps, hs.shape


def kernel(**inputs) -> np.ndarray:
    from concourse.bass_utils import run_bass_kernel_spmd

    in_maps, orig_shape = _make_in_maps(inputs)
    nc = _get_nc()
    res = run_bass_kernel_spmd(nc, in_maps, core_ids=list(range(N_CORES)))
    out = np.concatenate([res.results[c]["out"] for c in range(N_CORES)], axis=0)
    return out.reshape(orig_shape).astype(np.float32)


def run_traced(**inputs):
    """Like kernel() but returns (out, BassKernelResults with trace)."""
    from concourse.bass_utils import run_bass_kernel_spmd

    in_maps, orig_shape = _make_in_maps(inputs)
    nc = _get_nc()
    res = run_bass_kernel_spmd(
        nc, in_maps, core_ids=list(range(N_CORES)), trace=True
    )
    out = np.concatenate([res.results[c]["out"] for c in range(N_CORES)], axis=0)
    return out.reshape(orig_shape).astype(np.float32), res


# revision 14
# speedup vs baseline: 2.0749x; 2.0749x over previous
"""MoE routing kernel for TRN2, 8 NeuronCores (expert-parallel).

Math: out[t] = sum_{e in top2(logits[t])} x[t] @ w_up[e] @ w_down[e]
(reference applies no activation between projections and no prob weighting,
so each expert collapses to one fused matrix W_e = w_up[e] @ w_down[e]).

Distribution:
  phase 1 (expert parallel): core c computes W_c = w_up[c] @ w_down[c]
           in bf16 (fp32 PSUM accumulation), 4.3 G MACs/core.
  AllGather of the fused W matrices (bf16), split into two d2-halves so
           the collective overlaps phase-1 tail and apply head.
  phase 2 (token parallel): core c owns tokens [256c, 256(c+1)); computes
           fp32 routing logits, top-2 mask, and accumulates the 8 masked
           matmuls x_loc @ W_e into PSUM. Output is an exact token shard.
"""

import numpy as np

E = 8
TOPK = 2
D = 1024
I = 4096
T = 2048  # 4*512 tokens
N_CORES = 8
TL = T // N_CORES  # tokens per core (256)
P = 128
BIG = 1.0e30

_cached = {}


def _build():
    import concourse.bass as bass  # noqa: F401
    import concourse.tile as tile
    from concourse import bacc, mybir
    from concourse.masks import make_identity

    f32 = mybir.dt.float32
    bf16 = mybir.dt.bfloat16

    nc = bacc.Bacc("TRN2", target_bir_lowering=False, debug=False, num_devices=N_CORES)
    x_ext = nc.declare_dram_parameter("hidden_states", [TL, D], f32, isOutput=False)
    gate_ext = nc.declare_dram_parameter("gate_w", [E, D], f32, isOutput=False)
    up_ext = nc.declare_dram_parameter("w_up", [D, I], f32, isOutput=False)
    down_ext = nc.declare_dram_parameter("w_down", [I, D], f32, isOutput=False)
    out_ext = nc.declare_dram_parameter("out", [TL, D], f32, isOutput=True)

    KB = D // P      # 8  d1 blocks
    KI = I // P      # 32 i blocks
    TB = TL // P     # 2  token blocks
    NCH = D // 512   # 2  d2 chunks

    with tile.TileContext(nc) as tc:
        import contextlib

        with contextlib.ExitStack() as ctx:
            # ---- long-lived small pools ----
            const = ctx.enter_context(tc.tile_pool(name="const", bufs=1))
            outer = ctx.enter_context(tc.tile_pool(name="outer", bufs=1))
            dram = ctx.enter_context(tc.tile_pool(name="dram", bufs=1, space="DRAM"))

            ident = const.tile([P, P], f32)
            make_identity(nc, ident[:])
            ones1 = const.tile([1, P], f32)
            nc.vector.memset(ones1[:], 1.0)

            xT = outer.tile([P, KB, TL], f32)        # x_loc^T  [d1, t]
            mbc = outer.tile([P, E, TL], f32)        # per-expert masks bcast over partitions
            maskT = outer.tile([1, E, TL], f32)      # mask^T rows, all on partition 0
            mtmp = outer.tile([8, TB, P], f32)
            logits = outer.tile([P, TB, E], f32)
            m1 = outer.tile([P, TB], f32)
            eqbig = outer.tile([P, TB, E], f32)
            l2 = outer.tile([P, TB, E], f32)
            m2 = outer.tile([P, TB], f32)
            mask = outer.tile([P, TB, E], f32)
            gate_wT = outer.tile([P, KB, E], f32)

            # AG bounce buffers, one per d2 half; free dims flattened so every
            # DMA run is >=4KB contiguous per partition.
            ag_in = []
            ag_out = []
            for h in range(NCH):
                gi = dram.tile([P, KB * 512], bf16, name=f"ag_in_{h}")
                go = dram.tile(
                    [E, P, KB * 512], bf16, addr_space="Shared", name=f"ag_out_{h}"
                )
                ag_in.append(gi)
                ag_out.append(go)

            # ---- big weight pools (phase 1) ----
            big_cm = tc.tile_pool(name="big", bufs=1)
            big = big_cm.__enter__()
            upT = big.tile([P, KI, D], bf16)   # up^T  [i, d1]
            down = big.tile([P, KI, D], bf16)  # down  [i, d2]

            # ---- transient phase-0/1 pool ----
            ph1_cm = tc.tile_pool(name="ph1", bufs=3)
            ph1 = ph1_cm.__enter__()
            psA_cm = tc.tile_pool(name="psA", bufs=4, space="PSUM")
            psA = psA_cm.__enter__()

            # loads
            xnat = ph1.tile([P, TB, D], f32, tag="xnat", bufs=1)
            nc.sync.dma_start(xnat[:], x_ext.rearrange("(b p) d -> p b d", p=P))
            gsb = ph1.tile([8, D], f32, tag="gate", bufs=1)
            nc.sync.dma_start(gsb[:], gate_ext[:])

            # w_down cast-load (f32 -> bf16 via SWDGE), 4 quarters, 4KB runs
            for q in range(4):
                nc.gpsimd.dma_start(
                    down[:, 8 * q : 8 * (q + 1), :],
                    down_ext[1024 * q : 1024 * (q + 1), :].rearrange(
                        "(ko p) n -> p ko n", p=P
                    ),
                )

            # w_up: cast to a bf16 DRAM bounce (8 col-chunks), then 32 large
            # DRAM->SBUF xbar transpose loads (proven composable-matmul path).
            up_bf = dram.tile([D, I], bf16, name="up_bf")
            for cc in range(8):
                nc.gpsimd.dma_start(
                    up_bf[:, 512 * cc : 512 * (cc + 1)],
                    up_ext[:, 512 * cc : 512 * (cc + 1)],
                )
            for k in range(KI):
                nc.sync.dma_start_transpose(
                    upT[:, k, :], up_bf[:, P * k : P * (k + 1)]
                )

            # gate_w transpose: [8, 1024] -> [1024, 8]
            for kb in range(KB):
                pt = psA.tile([P, 8], f32, tag="tp")
                nc.tensor.transpose(pt[:], gsb[:, P * kb : P * (kb + 1)], ident[:8, :8])
                nc.any.tensor_copy(out=gate_wT[:, kb, :], in_=pt[:])

            # x transpose: [256, 1024] -> [1024, 256]
            for tb in range(TB):
                for kb in range(KB):
                    pt = psA.tile([P, P], f32, tag="tp")
                    nc.tensor.transpose(
                        pt[:], xnat[:, tb, P * kb : P * (kb + 1)], ident[:]
                    )
                    nc.any.tensor_copy(
                        out=xT[:, kb, P * tb : P * (tb + 1)], in_=pt[:]
                    )

            # router logits (fp32 exact): logits[t, e] = x @ gate_w.T
            for tb in range(TB):
                pl = psA.tile([P, E], f32, tag="tp")
                for kb in range(KB):
                    nc.tensor.matmul(
                        pl[:],
                        xT[:, kb, P * tb : P * (tb + 1)],
                        gate_wT[:, kb, :],
                        start=(kb == 0),
                        stop=(kb == KB - 1),
                    )
                nc.any.tensor_copy(out=logits[:, tb, :], in_=pl[:])

            # top-2 mask: mask = (l >= second_max(l))
            nc.vector.tensor_reduce(
                m1[:], logits[:], axis=mybir.AxisListType.X, op=mybir.AluOpType.max
            )
            nc.vector.tensor_tensor(
                eqbig[:],
                logits[:],
                m1[:, :, None].to_broadcast([P, TB, E]),
                mybir.AluOpType.is_equal,
            )
            nc.vector.tensor_scalar_mul(eqbig[:], eqbig[:], BIG)
            nc.vector.tensor_tensor(
                l2[:], logits[:], eqbig[:], mybir.AluOpType.subtract
            )
            nc.vector.tensor_reduce(
                m2[:], l2[:], axis=mybir.AxisListType.X, op=mybir.AluOpType.max
            )
            nc.vector.tensor_tensor(
                mask[:],
                logits[:],
                m2[:, :, None].to_broadcast([P, TB, E]),
                mybir.AluOpType.is_ge,
            )

            # mask^T: [256, 8] -> [8, 256], then DMA rows onto partition 0
            for tb in range(TB):
                pt = psA.tile([P, P], f32, tag="tp")
                nc.tensor.transpose(pt[:8, :], mask[:, tb, :], ident[:])
                nc.any.tensor_copy(out=mtmp[:, tb, :], in_=pt[:8, :])
            for tb in range(TB):
                nc.sync.dma_start(
                    maskT[0:1, :, P * tb : P * (tb + 1)], mtmp[:, tb, :]
                )

            # mask broadcast tiles via PE outer product (ones^T x maskT[e])
            for e in range(E):
                pb = psA.tile([P, TL], f32, tag="tp")
                nc.tensor.matmul(
                    pb[:], ones1[:], maskT[0:1, e, :], start=True, stop=True
                )
                nc.any.tensor_copy(out=mbc[:, e, :], in_=pb[:])

            psA_cm.__exit__(None, None, None)

            # ---- phase 1 matmuls: W_c = up^T.T @ down  (bf16) ----
            # nch-outer so the d2-half AllGather can fire at phase-1 midpoint.
            psW_cm = tc.tile_pool(name="psW", bufs=4, space="PSUM")
            psW = psW_cm.__enter__()
            for nch in range(NCH):
                for mg in range(2):
                    pw = [
                        psW.tile([P, 512], f32, tag="w", name=f"pw_{nch}_{mg}_{j}")
                        for j in range(4)
                    ]
                    for k in range(KI):
                        for m4 in range(4):
                            m = 4 * mg + m4
                            nc.tensor.matmul(
                                pw[m4][:],
                                upT[:, k, P * m : P * (m + 1)],
                                down[:, k, 512 * nch : 512 * (nch + 1)],
                                start=(k == 0),
                                stop=(k == KI - 1),
                            )
                    wev = ph1.tile([P, 4, 512], bf16, tag="wev")
                    for m4 in range(4):
                        nc.any.tensor_copy(out=wev[:, m4, :], in_=pw[m4][:])
                    nc.sync.dma_start(
                        ag_in[nch][:, 2048 * mg : 2048 * (mg + 1)], wev[:]
                    )
                # ---- AllGather for this d2 half ----
                nc.gpsimd.collective_compute(
                    "AllGather",
                    mybir.AluOpType.bypass,
                    replica_groups=[list(range(N_CORES))],
                    ins=[ag_in[nch].opt()],
                    outs=[ag_out[nch].opt()],
                )
            psW_cm.__exit__(None, None, None)

            # close phase-1 pools; open apply pools
            ph1_cm.__exit__(None, None, None)
            big_cm.__exit__(None, None, None)

            ap_cm = tc.tile_pool(name="apply", bufs=3)
            ap = ap_cm.__enter__()
            xmp_cm = tc.tile_pool(name="xm", bufs=1)
            xmp = xmp_cm.__enter__()
            psO_cm = tc.tile_pool(name="psO", bufs=4, space="PSUM")
            psO = psO_cm.__enter__()

            pout = [
                psO.tile([P, 512], f32, tag="o", name=f"pout_{j}") for j in range(4)
            ]
            xms = []

            for nch in range(NCH):
                for e in range(E):
                    we = ap.tile([P, KB * 512], bf16, tag="wstr", name=f"we_{nch}_{e}")
                    nc.sync.dma_start(we[:], ag_out[nch][e])
                    if nch == 0:
                        xm = xmp.tile([P, KB, TL], bf16, tag=f"xm{e}", name=f"xm_{e}")
                        for kb in range(KB):
                            nc.vector.tensor_tensor(
                                xm[:, kb, :],
                                xT[:, kb, :],
                                mbc[:, e, :],
                                mybir.AluOpType.mult,
                            )
                        xms.append(xm)
                    xm = xms[e]
                    for mt in range(TB):
                        for kb in range(KB):
                            nc.tensor.matmul(
                                pout[2 * mt + nch][:],
                                xm[:, kb, P * mt : P * (mt + 1)],
                                we[:, 512 * kb : 512 * (kb + 1)],
                                start=(e == 0 and kb == 0),
                                stop=(e == E - 1 and kb == KB - 1),
                            )

            outsb = ap.tile([P, TB, D], f32, tag="outsb", bufs=1)
            for mt in range(TB):
                for nch in range(NCH):
                    nc.any.tensor_copy(
                        out=outsb[:, mt, 512 * nch : 512 * (nch + 1)],
                        in_=pout[2 * mt + nch][:],
                    )
            nc.sync.dma_start(
                out_ext.rearrange("(b p) d -> p b d", p=P), outsb[:]
            )

            psO_cm.__exit__(None, None, None)
            xmp_cm.__exit__(None, None, None)
            ap_cm.__exit__(None, None, None)

    nc.finalize()
    return nc


def _get_nc():
    if "nc" not in _cached:
        _cached["nc"] = _build()
    return _cached["nc"]


def _make_in_maps(inputs):
    hs = np.asarray(inputs["hidden_states"], dtype=np.float32)
    gate_w = np.ascontiguousarray(np.asarray(inputs["gate_w"], dtype=np.float32))
    w_up = np.asarray(inputs["w_up"], dtype=np.float32)
    w_down = np.asarray(inputs["w_down"], dtype=np.float32)
    x = np.ascontiguousarray(hs.reshape(-1, D))
    in_maps = []
    for c in range(N_CORES):
        in_maps.append(
            {
                "hidden_states": np.ascontiguousarray(x[TL * c : TL * (c + 1)]),
                "gate_w": gate_w,
                "w_up": np.ascontiguousarray(w_up[c]),
                "w_down": np.ascontiguousarray(w_down[c]),
            }
        )
    return in_maps, hs.shape


def kernel(**inputs) -> np.ndarray:
    from concourse.bass_utils import run_bass_kernel_spmd

    in_maps, orig_shape = _make_in_maps(inputs)
    nc = _get_nc()
    res = run_bass_kernel_spmd(nc, in_maps, core_ids=list(range(N_CORES)))
    out = np.concatenate([res.results[c]["out"] for c in range(N_CORES)], axis=0)
    return out.reshape(orig_shape).astype(np.float32)


def run_traced(**inputs):
    """Like kernel() but returns (out, BassKernelResults with trace)."""
    from concourse.bass_utils import run_bass_kernel_spmd

    in_maps, orig_shape = _make_in_maps(inputs)
    nc = _get_nc()
    res = run_bass_kernel_spmd(
        nc, in_maps, core_ids=list(range(N_CORES)), trace=True
    )
    out = np.concatenate([res.results[c]["out"] for c in range(N_CORES)], axis=0)
    return out.reshape(orig_shape).astype(np.float32), res
